# revision 1
# baseline (speedup 1.0000x reference)
"""DPQ embedding (vq_codebook) Trainium2 kernel.

Computes, for inputs ids[32,2048], query_wemb[100000,512], centroids[8,256,64]:
  x = wemb[ids]  -> [N, 8, 64]
  response[n,d,k] = -||x_nd||^2 + 2 x_nd.c_dk - ||c_dk||^2
  BN over (n,d) per k (training stats), argmax_k, gather centroids -> [N, 512]

Strategy: data-parallel over tokens on 8 cores; embedding table replicated
(augmented on host to [vocab, 528] with baked ones/h columns, gathered by
indirect DMA); BN statistics computed exactly via per-subspace Gram matrices
G = Y^T Y (Y = [x | 1 | h], h = ||x||^2) accumulated on PE and AllReduced
(139KB) -- responses are never materialized; normalized responses
z = s_k*(r - m_k) come from a single fp32 matmul per (tile, d) with augmented
66-row centroid matrices (scale/beta/h-coefficient folded in); argmax via a
DVE prefix-max scan + ACT Sign-with-accumulator counting strict prefixes
below the row max (first-occurrence argmax, exact in fp32); the tiny code
tensor [N, 8] is returned and the final centroid row lookup happens on host.
The straight-through estimator (out - x) + x is the identity in the forward
pass up to 1-ulp rounding and is omitted.

A post-scheduling pass (_hoist_excess_waits) splits semaphore waits onto
standalone EventSemaphore instructions because this walrus build rejects >1
sync-wait command per compute instruction and any wait on a Drain.
"""

import os
import sys

for _p in ("/opt/trn_rl_repo", "/root/.axon_site/_ro/trn_rl_repo"):
    if os.path.isdir(_p) and _p not in sys.path:
        sys.path.insert(0, _p)
        break

from contextlib import ExitStack

import ml_dtypes
import numpy as np

import concourse.bass as bass
import concourse.tile as tile
from concourse import mybir
from concourse.masks import make_identity

VOCAB = 100000
EMB = 512
D = 8
K = 256
SUB = 64
AUG = SUB + 2  # 66: [x(64) | ones | h]
WAUG = D * AUG  # 528
BN_EPS = 1e-3
P = 128

F32 = mybir.dt.float32
BF16 = mybir.dt.bfloat16
I32 = mybir.dt.int32


def _hoist_excess_waits(nc, cap=1):
    """This walrus build rejects instructions carrying too many sync-wait
    commands (and any wait on a Drain). Hoist excess waits into standalone
    InstEventSemaphore instructions right before the offender, same engine."""
    uid = 0
    for f in nc.m.functions:
        for b in f.blocks:
            insts = b.instructions
            i = 0
            while i < len(insts):
                inst = insts[i]
                si = inst.sync_info
                if si is not None and si.on_wait:
                    c = 0 if type(inst).__name__ == "InstDrain" else cap
                    waits = list(si.on_wait)
                    if len(waits) > c:
                        nh = len(waits) - c
                        for w in waits[:nh]:
                            uid += 1
                            ev = mybir.InstEventSemaphore(
                                name=f"EVW-{uid}",
                                engine=inst.engine,
                                ins=[],
                                outs=[],
                                sync_info=mybir.SyncInfo(on_wait=[w], on_update=[]),
                            )
                            insts.insert(i, ev)
                            i += 1
                        inst.sync_info = mybir.SyncInfo(
                            on_wait=waits[nh:], on_update=list(si.on_update)
                        )
                i += 1
    return nc


def build(npc, ncores, vocab=VOCAB, debug=False):
    """Build the SPMD Bass program for `npc` tokens per core."""
    nt = npc // P  # token tiles per core
    nd_tot = npc * ncores * D  # BN sample count

    nc = bass.Bass()
    dbg = {}
    if debug:
        for nm, shp in [
            ("dbg_g", [AUG, WAUG]),
            ("dbg_sumr", [1, K]),
            ("dbg_ssum", [1, K]),
            ("dbg_sh1", [1, D]),
            ("dbg_shh1", [1, D]),
            ("dbg_mean", [1, K]),
            ("dbg_var", [1, K]),
            ("dbg_caug", [AUG, D * K]),
            ("dbg_acc0", [P, D]),
            ("dbg_pscan00", [P, K]),
            ("dbg_z00", [P, K]),
        ]:
            dbg[nm] = nc.dram_tensor(nm, shp, F32, kind="ExternalOutput")

    table = nc.dram_tensor("table", [vocab, WAUG], F32, kind="ExternalInput")
    ct = nc.dram_tensor("ct", [SUB, D * K], F32, kind="ExternalInput")
    c2pd = nc.dram_tensor("c2pd", [D, K], F32, kind="ExternalInput")
    ids = nc.dram_tensor("ids", [P, nt], I32, kind="ExternalInput")
    out = nc.dram_tensor("out", [npc, D], F32, kind="ExternalOutput")

    g_loc = nc.dram_tensor("g_loc", [AUG, WAUG], F32)
    g_sum = nc.dram_tensor(
        "g_sum", [AUG, WAUG], F32, addr_space="Shared" if ncores > 4 else "Local"
    )

    with ExitStack() as ctx:
        tc = ctx.enter_context(tile.TileContext(nc))
        con = ctx.enter_context(tc.tile_pool(name="con", bufs=1))
        xap = ctx.enter_context(tc.tile_pool(name="xap", bufs=1))
        wrk = ctx.enter_context(tc.tile_pool(name="wrk", bufs=2))
        msk = ctx.enter_context(tc.tile_pool(name="msk", bufs=3))
        obp = ctx.enter_context(tc.tile_pool(name="obp", bufs=3))
        pg = ctx.enter_context(tc.tile_pool(name="pg", bufs=1, space="PSUM"))
        pxt = ctx.enter_context(tc.tile_pool(name="pxt", bufs=2, space="PSUM"))
        pz = ctx.enter_context(tc.tile_pool(name="pz", bufs=4, space="PSUM"))
        pbn = pz  # BN-block psum tiles reuse the z-pool slots (tag-shared)

        # ---- constants / small inputs ----
        ident = con.tile([P, P], F32)
        make_identity(nc, ident[:])
        ones64 = con.tile([SUB, 1], F32)
        nc.gpsimd.memset(ones64[:], 1.0)
        ids_sb = con.tile([P, nt], I32)
        nc.sync.dma_start(ids_sb[:], ids[:])
        c2pd_sb = con.tile([D, K], F32)
        nc.sync.dma_start(c2pd_sb[:], c2pd[:])
        zf_sb = con.tile([P, K], F32)
        nc.gpsimd.memset(zf_sb[:], 0.0)

        # ---- phase A: gather + h + Gram accumulation ----
        xa = []
        for t in range(nt):
            xt = xap.tile([P, WAUG], F32, tag=f"xa{t}")
            xa.append(xt)
            nc.gpsimd.indirect_dma_start(
                out=xt[:],
                out_offset=None,
                in_=table[:],
                in_offset=bass.IndirectOffsetOnAxis(ap=ids_sb[:, t : t + 1], axis=0),
            )
            # h[n,d] = sum_s x^2: square on gpsimd, 3D-reduce + write on DVE
            xv = xt[:].rearrange("p (d c) -> p d c", c=AUG)[:, :, 0:SUB]
            x2 = wrk.tile([P, D * SUB], F32, tag="x2")
            x2v = x2[:].rearrange("p (d c) -> p d c", c=SUB)
            nc.gpsimd.tensor_tensor(out=x2v, in0=xv, in1=xv, op=mybir.AluOpType.mult)
            htmp = wrk.tile([P, D], F32, tag="htmp")
            nc.vector.tensor_reduce(
                out=htmp[:], in_=x2v, axis=mybir.AxisListType.X, op=mybir.AluOpType.add
            )
            hcols = xt[:].rearrange("p (d c) -> p d c", c=AUG)[:, :, SUB + 1 : SUB + 2]
            nc.vector.tensor_copy(hcols, htmp[:])

        g_sb = con.tile([AUG, WAUG], F32)
        for d in range(D):
            gp = pg.tile([AUG, AUG], F32, tag=f"gb{d % 2}", name="gp")
            for t in range(nt):
                nc.tensor.matmul(
                    gp[:],
                    lhsT=xa[t][:, AUG * d : AUG * d + AUG],
                    rhs=xa[t][:, AUG * d : AUG * d + AUG],
                    start=(t == 0),
                    stop=(t == nt - 1),
                )
            nc.scalar.activation(
                g_sb[:, AUG * d : AUG * d + AUG],
                gp[:],
                mybir.ActivationFunctionType.Copy,
            )
        nc.sync.dma_start(g_loc[:], g_sb[:])
        nc.gpsimd.collective_compute(
            "AllReduce",
            mybir.AluOpType.add,
            replica_groups=[list(range(ncores))],
            ins=[g_loc[:]],
            outs=[g_sum[:]],
        )
        nc.sync.dma_start(g_sb[:], g_sum[:])

        # ---- BN parameter block (small) ----
        # per-d scalars: sh_d = sum h, shh_d = sum h^2 (k-free)
        gv = g_sb[:].rearrange("p (d c) -> p d c", c=AUG)
        sh1 = con.tile([1, D], F32)
        nc.sync.dma_start(sh1[:], gv[SUB : SUB + 1, :, SUB + 1 : SUB + 2])
        shh1 = con.tile([1, D], F32)
        nc.sync.dma_start(shh1[:], gv[SUB + 1 : SUB + 2, :, SUB + 1 : SUB + 2])
        shhtot = con.tile([1, 1], F32)
        nc.vector.reduce_sum(shhtot[:], shh1[:], axis=mybir.AxisListType.X)

        sh2 = con.tile([1, D], F32)
        nc.vector.tensor_scalar_mul(sh2[:], sh1[:], 2.0)

        ntf = float(npc * ncores)  # token count for c2 scaling
        # accumulate over d:
        #   sumr[k] += 2*u_dk - NT*c2_dk - sh_d
        #   ssum[k] += 4*(Pq_dk - w_dk) + c2_dk*(NT*c2_dk - 4*u_dk + 2*sh_d)
        sumr = con.tile([1, K], F32)
        nc.gpsimd.memset(sumr[:], 0.0)
        ssum = con.tile([1, K], F32)
        nc.gpsimd.memset(ssum[:], 0.0)
        for d in range(D):
            ct_d = wrk.tile([SUB, K], F32, tag="ctd")
            nc.sync.dma_start(ct_d[:], ct[:, K * d : K * d + K])
            c2_d = wrk.tile([1, K], F32, tag="c2d")
            nc.sync.dma_start(c2_d[:], c2pd[d : d + 1, :])
            t_ps = pbn.tile([SUB, K], F32, tag="zps")
            nc.tensor.matmul(
                t_ps[:], lhsT=g_sb[0:SUB, AUG * d : AUG * d + SUB], rhs=ct_d[:]
            )
            m_sb = wrk.tile([SUB, K], F32, tag="msb")
            nc.vector.tensor_tensor(
                out=m_sb[:], in0=ct_d[:], in1=t_ps[:], op=mybir.AluOpType.mult
            )
            pq_ps = pbn.tile([1, K], F32, tag="zps")
            nc.tensor.matmul(pq_ps[:], lhsT=ones64[:], rhs=m_sb[:])
            u_ps = pbn.tile([1, K], F32, tag="zps", name="u_ps")
            nc.tensor.matmul(
                u_ps[:],
                lhsT=g_sb[0:SUB, AUG * d + SUB : AUG * d + SUB + 1],
                rhs=ct_d[:],
            )
            w_ps = pbn.tile([1, K], F32, tag="zps", name="w_ps")
            nc.tensor.matmul(
                w_ps[:],
                lhsT=g_sb[0:SUB, AUG * d + SUB + 1 : AUG * d + AUG],
                rhs=ct_d[:],
            )
            # sumr += (u*2 - sh_d) + c2*(-NT)
            t1 = wrk.tile([1, K], F32, tag="t1")
            nc.vector.tensor_scalar(
                out=t1[:],
                in0=u_ps[:],
                scalar1=2.0,
                scalar2=sh1[:, d : d + 1],
                op0=mybir.AluOpType.mult,
                op1=mybir.AluOpType.subtract,
            )
            t2 = wrk.tile([1, K], F32, tag="t2")
            nc.vector.tensor_scalar_mul(t2[:], c2_d[:], -ntf)
            nc.vector.tensor_tensor(
                out=t1[:], in0=t1[:], in1=t2[:], op=mybir.AluOpType.add
            )
            nc.vector.tensor_tensor(
                out=sumr[:], in0=sumr[:], in1=t1[:], op=mybir.AluOpType.add
            )
            # f = (u*(-4) + 2sh_d - t2) * c2 ; e = 4*(Pq - w) ; ssum += e + f
            f = wrk.tile([1, K], F32, tag="f")
            nc.vector.tensor_scalar(
                out=f[:],
                in0=u_ps[:],
                scalar1=-4.0,
                scalar2=sh2[:, d : d + 1],
                op0=mybir.AluOpType.mult,
                op1=mybir.AluOpType.add,
            )
            nc.vector.tensor_tensor(
                out=f[:], in0=f[:], in1=t2[:], op=mybir.AluOpType.subtract
            )
            nc.vector.tensor_tensor(
                out=f[:], in0=f[:], in1=c2_d[:], op=mybir.AluOpType.mult
            )
            e = wrk.tile([1, K], F32, tag="e")
            nc.vector.tensor_scalar_mul(e[:], w_ps[:], -4.0)
            nc.vector.tensor_tensor(
                out=e[:], in0=e[:], in1=f[:], op=mybir.AluOpType.add
            )
            t3 = wrk.tile([1, K], F32, tag="t3")
            nc.vector.tensor_scalar_mul(t3[:], pq_ps[:], 4.0)
            nc.vector.tensor_tensor(
                out=e[:], in0=e[:], in1=t3[:], op=mybir.AluOpType.add
            )
            nc.vector.tensor_tensor(
                out=ssum[:], in0=ssum[:], in1=e[:], op=mybir.AluOpType.add
            )
        nc.vector.tensor_scalar(
            out=ssum[:],
            in0=ssum[:],
            scalar1=shhtot[:, 0:1],
            scalar2=None,
            op0=mybir.AluOpType.add,
        )
        # mean, var, s
        inv_nd = 1.0 / float(nd_tot)
        mean = con.tile([1, K], F32)
        nc.vector.tensor_scalar_mul(mean[:], sumr[:], inv_nd)
        var = con.tile([1, K], F32)
        nc.vector.tensor_scalar_mul(var[:], ssum[:], inv_nd)
        m2 = con.tile([1, K], F32)
        nc.vector.tensor_tensor(
            out=m2[:], in0=mean[:], in1=mean[:], op=mybir.AluOpType.mult
        )
        nc.vector.tensor_tensor(
            out=var[:], in0=var[:], in1=m2[:], op=mybir.AluOpType.subtract
        )
        if debug:
            nc.sync.dma_start(dbg["dbg_g"][:], g_sb[:])
            nc.sync.dma_start(dbg["dbg_sumr"][:], sumr[:])
            nc.sync.dma_start(dbg["dbg_ssum"][:], ssum[:])
            nc.sync.dma_start(dbg["dbg_sh1"][:], sh1[:])
            nc.sync.dma_start(dbg["dbg_shh1"][:], shh1[:])
            nc.sync.dma_start(dbg["dbg_mean"][:], mean[:])
            nc.sync.dma_start(dbg["dbg_var"][:], var[:])
        nc.vector.tensor_scalar_add(var[:], var[:], BN_EPS)
        rec = con.tile([1, K], F32)
        nc.vector.reciprocal(rec[:], var[:])
        sca = con.tile([1, K], F32)
        nc.scalar.activation(sca[:], rec[:], mybir.ActivationFunctionType.Sqrt)
        nsca = con.tile([1, K], F32)
        nc.vector.tensor_scalar_mul(nsca[:], sca[:], -1.0)
        s2 = con.tile([1, K], F32)
        nc.vector.tensor_scalar_mul(s2[:], sca[:], 2.0)
        # materialized partition-broadcasts of the [1, K] rows
        # (outer product ones[SUB] x row[K] on the PE)
        ones_row = con.tile([1, SUB], F32)
        nc.gpsimd.memset(ones_row[:], 1.0)
        meanb = con.tile([SUB, K], F32)
        nscab = con.tile([SUB, K], F32)
        s2b = con.tile([SUB, K], F32)
        for src, dst in ((mean, meanb), (nsca, nscab), (s2, s2b)):
            bc_ps = pbn.tile([SUB, K], F32, tag="zps", name="bc_ps")
            nc.tensor.matmul(bc_ps[:], lhsT=ones_row[:], rhs=src[:])
            nc.scalar.activation(
                dst[:], bc_ps[:], mybir.ActivationFunctionType.Copy
            )
        # beta[d,k] = -(c2 + mean) * s
        beta = con.tile([D, K], F32)
        nc.vector.tensor_tensor(
            out=beta[:], in0=c2pd_sb[:], in1=meanb[0:D, :], op=mybir.AluOpType.add
        )
        nc.vector.tensor_tensor(
            out=beta[:], in0=beta[:], in1=nscab[0:D, :], op=mybir.AluOpType.mult
        )
        # caug[66, K] per d: rows 0:64 = 2*s*c^T, row 64 = beta, row 65 = -s
        caug = con.tile([AUG, D * K], F32)
        for d in range(D):
            ct_d2 = wrk.tile([SUB, K], F32, tag="ctd")
            nc.sync.dma_start(ct_d2[:], ct[:, K * d : K * d + K])
            nc.vector.tensor_tensor(
                out=caug[0:SUB, K * d : K * d + K],
                in0=ct_d2[:],
                in1=s2b[:],
                op=mybir.AluOpType.mult,
            )
            nc.sync.dma_start(
                caug[SUB : SUB + 1, K * d : K * d + K], beta[d : d + 1, :]
            )
            nc.sync.dma_start(
                caug[SUB + 1 : SUB + 2, K * d : K * d + K], nsca[0:1, :]
            )

        # ---- phase B: transpose, z, argmax, gather ----
        for t in range(nt):
            xt = xa[t]
            xt_ps = [pxt.tile([AUG, 4 * P], F32, tag="xtps", name="xt_ps") for _ in range(2)]
            for d in range(D):
                nc.tensor.transpose(
                    out=xt_ps[d // 4][:, P * (d % 4) : P * (d % 4) + P],
                    in_=xt[:, AUG * d : AUG * d + AUG],
                    identity=ident[:],
                )
            xt_sb = [wrk.tile([AUG, 4 * P], F32, tag="xtsb", name="xt_sb") for _ in range(2)]
            for i in range(2):
                nc.scalar.activation(
                    xt_sb[i][:], xt_ps[i][:], mybir.ActivationFunctionType.Copy
                )
            zps = [pz.tile([P, 2 * K], F32, tag="zps", name="zps") for _ in range(4)]
            for d in range(D):
                nc.tensor.matmul(
                    zps[d // 2][:, K * (d % 2) : K * (d % 2) + K],
                    lhsT=xt_sb[d // 4][:, P * (d % 4) : P * (d % 4) + P],
                    rhs=caug[:, K * d : K * d + K],
                )
            # argmax via prefix-max scan: k* = sum_k 1[pscan_k < rowmax],
            # rowmax = pscan[:, K-1]. Scan on DVE, sign+accumulate on ACT.
            acc = msk.tile([P, D], F32, tag="acc")
            for d in range(D):
                pscan = msk.tile([P, K], F32, tag="pscan")
                nc.vector.tensor_tensor_scan(
                    out=pscan[:],
                    data0=zps[d // 2][:, K * (d % 2) : K * (d % 2) + K],
                    data1=zf_sb[:],
                    initial=-1e30,
                    op0=mybir.AluOpType.max,
                    op1=mybir.AluOpType.bypass,
                )
                dum = msk.tile([P, K], BF16, tag="dum")
                nc.scalar.activation(
                    dum[:],
                    pscan[:],
                    mybir.ActivationFunctionType.Sign,
                    bias=pscan[:, K - 1 : K],
                    scale=-1.0,
                    accum_out=acc[:, d : d + 1],
                )
                if debug and t == 0 and d == 0:
                    nc.sync.dma_start(dbg["dbg_pscan00"][:], pscan[:])
                    ztmp = msk.tile([P, K], F32, tag="ztmp", name="ztmp")
                    nc.vector.tensor_copy(ztmp[:], zps[0][:, 0:K])
                    nc.sync.dma_start(dbg["dbg_z00"][:], ztmp[:])
            if debug and t == 0:
                nc.sync.dma_start(dbg["dbg_acc0"][:], acc[:])
            nc.sync.dma_start(out[P * t : P * t + P, :], acc[:])

    return nc


def prep_host(query_wemb, centroids):
    """Host-side layout prep (pure functions of the weights)."""
    vocab = query_wemb.shape[0]
    table = np.zeros((vocab, WAUG), dtype=np.float32)
    tv = table.reshape(vocab, D, AUG)
    tv[:, :, 0:SUB] = query_wemb.reshape(vocab, D, SUB)
    tv[:, :, SUB] = 1.0  # ones column (the h column stays 0; filled on device)
    ct = np.ascontiguousarray(
        centroids.transpose(0, 2, 1).reshape(D, SUB, K).transpose(1, 0, 2).reshape(SUB, D * K)
    )
    # ct[s, d*K + k] = centroids[d, k, s]
    c2pd = np.sum(centroids.astype(np.float64) ** 2, axis=-1).astype(np.float32)  # [D,K]
    return dict(table=table, ct=ct, c2pd=c2pd)


def make_in_maps(inputs, query_wemb, centroids, ncores):
    common = prep_host(np.asarray(query_wemb), np.asarray(centroids))
    ids_all = np.asarray(inputs, dtype=np.int32).reshape(-1)
    npc = ids_all.size // ncores
    nt = npc // P
    in_maps = []
    for c in range(ncores):
        ids_c = ids_all[c * npc : (c + 1) * npc].reshape(nt, P).T.copy()
        in_maps.append({**common, "ids": ids_c})
    return in_maps, npc


_CACHE = {}


def kernel(inputs, query_wemb, centroids):
    from concourse.bass_utils import run_bass_kernel_spmd

    inputs = np.asarray(inputs)
    ncores = 8
    in_maps, npc = make_in_maps(inputs, query_wemb, centroids, ncores)
    key = (npc, ncores)
    if key not in _CACHE:
        _CACHE[key] = _hoist_excess_waits(
            build(npc, ncores, vocab=np.asarray(query_wemb).shape[0])
        )
    nc = _CACHE[key]
    res = run_bass_kernel_spmd(nc, in_maps, list(range(ncores)))
    codes = np.concatenate(
        [res.results[c]["out"] for c in range(ncores)], axis=0
    )  # [N, D] float32 exact integers
    codes = np.rint(codes).astype(np.int64)
    cent = np.asarray(centroids, dtype=np.float32)  # [D, K, SUB]
    full = cent[np.arange(D)[None, :], codes]  # [N, D, SUB]
    return (
        full.reshape(inputs.shape + (EMB,)).astype(np.float32)
    )



# revision 10
# speedup vs baseline: 25.8093x; 25.8093x over previous
"""DPQ embedding (vq_codebook) Trainium2 kernel, v3.

Computes, for inputs ids[32,2048], query_wemb[100000,512], centroids[8,256,64]:
  x = wemb[ids]  -> [N, 8, 64]
  response[n,d,k] = -||x_nd||^2 + 2 x_nd.c_dk - ||c_dk||^2
  BN over (n,d) per k (training stats), argmax_k, gather centroids -> [N, 512]

Sharding strategy (v3): data-parallel over tokens on 8 cores. The host
performs the embedding-row gather (it holds both ids and the table), so each
core uploads only the 8192x512 fp32 rows it needs (17 MB/core, 134 MB total)
instead of a replicated 211 MB table per core (1.7 GB) -- host->device I/O is
the wall-clock bottleneck under this axon runtime. On device, a strided DMA
widens each row tile into the augmented layout [x(64) | 1 | h] per subspace;
h is filled on device. BN statistics are computed exactly via per-subspace
Gram matrices G_d = Y_d^T Y_d accumulated on the PE across all tiles (8
accumulation groups packed into 2 PSUM banks, overlapping the input DMAs),
reduced to per-k sums locally, and AllReduced as a single [2, 256] tensor
(2 KB, vs 139 KB in v2). Normalized responses z = s_k*(r - m_k) come from one
fp32 matmul per (tile, d) with augmented 66-row centroid matrices
(scale/beta/h-coefficient folded in); argmax via a DVE prefix-max scan + ACT
Sign-with-accumulator counting strict prefixes below the row max
(first-occurrence argmax, exact in fp32); the tiny code tensor [N, 8] is
returned and the final centroid row lookup happens on host. The
straight-through estimator (out - x) + x is the identity in the forward pass
and is omitted.

A post-scheduling pass (_hoist_excess_waits) splits semaphore waits onto
standalone EventSemaphore instructions because this walrus build rejects >1
sync-wait command per compute instruction and any wait on a Drain.
"""

import os
import sys

for _p in ("/opt/trn_rl_repo", "/root/.axon_site/_ro/trn_rl_repo"):
    if os.path.isdir(_p) and _p not in sys.path:
        sys.path.insert(0, _p)
        break

from contextlib import ExitStack

import numpy as np

import concourse.bass as bass
import concourse.tile as tile
from concourse import mybir
from concourse.masks import make_identity

VOCAB = 100000
EMB = 512
D = 8
K = 256
SUB = 64
AUG = SUB + 2  # 66: [x(64) | ones | h]
WAUG = D * AUG  # 528
BN_EPS = 1e-3
P = 128

F32 = mybir.dt.float32
BF16 = mybir.dt.bfloat16
I32 = mybir.dt.int32


def _hoist_excess_waits(nc, cap=1):
    """This walrus build rejects instructions carrying too many sync-wait
    commands (and any wait on a Drain). Hoist excess waits into standalone
    InstEventSemaphore instructions right before the offender, same engine."""
    uid = 0
    for f in nc.m.functions:
        for b in f.blocks:
            insts = b.instructions
            i = 0
            while i < len(insts):
                inst = insts[i]
                si = inst.sync_info
                if si is not None and si.on_wait:
                    c = 0 if type(inst).__name__ == "InstDrain" else cap
                    waits = list(si.on_wait)
                    if len(waits) > c:
                        nh = len(waits) - c
                        for w in waits[:nh]:
                            uid += 1
                            ev = mybir.InstEventSemaphore(
                                name=f"EVW-{uid}",
                                engine=inst.engine,
                                ins=[],
                                outs=[],
                                sync_info=mybir.SyncInfo(on_wait=[w], on_update=[]),
                            )
                            insts.insert(i, ev)
                            i += 1
                        inst.sync_info = mybir.SyncInfo(
                            on_wait=waits[nh:], on_update=list(si.on_update)
                        )
                i += 1
    return nc


def build(npc, ncores, debug=False):
    """Build the SPMD Bass program for `npc` tokens per core."""
    nt = npc // P  # token tiles per core
    nd_tot = npc * ncores * D  # BN sample count

    nc = bass.Bass()
    dbg = {}
    if debug:
        for nm, shp in [
            ("dbg_g", [AUG, WAUG]),
            ("dbg_red", [2, K]),
            ("dbg_mean", [1, K]),
            ("dbg_var", [1, K]),
            ("dbg_caug", [AUG, D * K]),
            ("dbg_z00", [P, K]),
        ]:
            dbg[nm] = nc.dram_tensor(nm, shp, F32, kind="ExternalOutput")

    xg = nc.dram_tensor("xg", [npc, EMB], F32, kind="ExternalInput")
    ct = nc.dram_tensor("ct", [SUB, D * K], F32, kind="ExternalInput")
    c2pd = nc.dram_tensor("c2pd", [D, K], F32, kind="ExternalInput")
    out = nc.dram_tensor("out", [npc, D], F32, kind="ExternalOutput")

    red_loc = nc.dram_tensor("red_loc", [2, K], F32)
    red_sum = nc.dram_tensor(
        "red_sum", [2, K], F32, addr_space="Shared" if ncores > 4 else "Local"
    )

    with ExitStack() as ctx:
        tc = ctx.enter_context(tile.TileContext(nc))
        con = ctx.enter_context(tc.tile_pool(name="con", bufs=1))
        xap = ctx.enter_context(tc.tile_pool(name="xap", bufs=1))
        wrk = ctx.enter_context(tc.tile_pool(name="wrk", bufs=2))
        xts = ctx.enter_context(tc.tile_pool(name="xts", bufs=3))
        msk = ctx.enter_context(tc.tile_pool(name="msk", bufs=3))
        pg = ctx.enter_context(tc.tile_pool(name="pg", bufs=1, space="PSUM"))
        pxt = ctx.enter_context(tc.tile_pool(name="pxt", bufs=2, space="PSUM"))
        pz = ctx.enter_context(tc.tile_pool(name="pz", bufs=1, space="PSUM"))

        # ---- constants / small inputs ----
        ident = con.tile([P, P], F32)
        make_identity(nc, ident[:])
        ones64 = con.tile([SUB, 1], F32)
        nc.gpsimd.memset(ones64[:], 1.0)
        c2pd_sb = con.tile([D, K], F32)
        nc.sync.dma_start(c2pd_sb[:], c2pd[:])
        ct_sb = con.tile([SUB, D * K], F32)
        nc.sync.dma_start(ct_sb[:], ct[:])
        zf_sb = con.tile([P, K], F32)
        nc.gpsimd.memset(zf_sb[:], 0.0)

        # ---- phase A: widen-load + h, then d-major Gram accumulation ----
        # (PSUM accumulation groups own a full 2KB zero region, so the 8
        # Gram groups must run sequentially; the d=0 sweep visits tiles in
        # DMA arrival order, so it still overlaps the input loads.)
        xa = []
        for t in range(nt):
            xt = xap.tile([P, WAUG], F32, tag=f"xa{t}")
            xa.append(xt)
            xv3 = xt[:].rearrange("p (d c) -> p d c", c=AUG)
            nc.sync.dma_start(
                xv3[:, :, 0:SUB],
                xg[P * t : P * t + P, :].rearrange("p (d c) -> p d c", c=SUB),
            )
            nc.gpsimd.memset(xv3[:, :, SUB : SUB + 1], 1.0)
            xv = xv3[:, :, 0:SUB]
            x2 = wrk.tile([P, D * SUB], F32, tag="x2")
            x2v = x2[:].rearrange("p (d c) -> p d c", c=SUB)
            nc.gpsimd.tensor_tensor(out=x2v, in0=xv, in1=xv, op=mybir.AluOpType.mult)
            htmp = wrk.tile([P, D], F32, tag="htmp")
            nc.vector.tensor_reduce(
                out=htmp[:], in_=x2v, axis=mybir.AxisListType.X, op=mybir.AluOpType.add
            )
            nc.vector.tensor_copy(xv3[:, :, SUB + 1 : SUB + 2], htmp[:])

        g_sb = con.tile([AUG, WAUG], F32)
        for d in range(D):
            gp = pg.tile([AUG, AUG], F32, tag=f"gb{d % 2}", name="gp")
            for t in range(nt):
                nc.tensor.matmul(
                    gp[:],
                    lhsT=xa[t][:, AUG * d : AUG * d + AUG],
                    rhs=xa[t][:, AUG * d : AUG * d + AUG],
                    start=(t == 0),
                    stop=(t == nt - 1),
                )
            nc.scalar.activation(
                g_sb[:, AUG * d : AUG * d + AUG],
                gp[:],
                mybir.ActivationFunctionType.Copy,
            )

        # ---- local BN sums, then a tiny AllReduce ----
        # per-d scalars: sh_d = sum h, shh_d = sum h^2 (k-free)
        gv = g_sb[:].rearrange("p (d c) -> p d c", c=AUG)
        sh1 = con.tile([1, D], F32)
        nc.sync.dma_start(sh1[:], gv[SUB : SUB + 1, :, SUB + 1 : SUB + 2])
        shh1 = con.tile([1, D], F32)
        nc.sync.dma_start(shh1[:], gv[SUB + 1 : SUB + 2, :, SUB + 1 : SUB + 2])
        shhtot = con.tile([1, 1], F32)
        nc.vector.reduce_sum(shhtot[:], shh1[:], axis=mybir.AxisListType.X)

        sh2 = con.tile([1, D], F32)
        nc.vector.tensor_scalar_mul(sh2[:], sh1[:], 2.0)

        ntf = float(npc)  # LOCAL token count (sums are AllReduced below)
        # accumulate over d into red2 rows:
        #   row0 sumr[k] += 2*u_dk - NT*c2_dk - sh_d
        #   row1 ssum[k] += 4*(Pq_dk - w_dk) + c2_dk*(NT*c2_dk - 4*u_dk + 2*sh_d)
        sumr_t = con.tile([1, K], F32)
        ssum_t = con.tile([1, K], F32)
        sumr = sumr_t[:]
        ssum = ssum_t[:]
        nc.gpsimd.memset(sumr, 0.0)
        nc.gpsimd.memset(ssum, 0.0)
        for d in range(D):
            ct_d = ct_sb[:, K * d : K * d + K]
            c2_t = wrk.tile([1, K], F32, tag="c2d")
            nc.sync.dma_start(c2_t[:], c2pd[d : d + 1, :])
            c2_d = c2_t[:]
            t_ps = pz.tile([SUB, K], F32, tag="za", name="t_ps")
            nc.tensor.matmul(
                t_ps[:], lhsT=g_sb[0:SUB, AUG * d : AUG * d + SUB], rhs=ct_d
            )
            m_sb = wrk.tile([SUB, K], F32, tag="msb")
            nc.vector.tensor_tensor(
                out=m_sb[:], in0=ct_d, in1=t_ps[:], op=mybir.AluOpType.mult
            )
            pq_ps = pz.tile([1, K], F32, tag="zb", name="pq_ps")
            nc.tensor.matmul(pq_ps[:], lhsT=ones64[:], rhs=m_sb[:])
            u_ps = pz.tile([1, K], F32, tag="za", name="u_ps")
            nc.tensor.matmul(
                u_ps[:],
                lhsT=g_sb[0:SUB, AUG * d + SUB : AUG * d + SUB + 1],
                rhs=ct_d,
            )
            w_ps = pz.tile([1, K], F32, tag="zb", name="w_ps")
            nc.tensor.matmul(
                w_ps[:],
                lhsT=g_sb[0:SUB, AUG * d + SUB + 1 : AUG * d + AUG],
                rhs=ct_d,
            )
            # sumr += (u*2 - sh_d) + c2*(-NT)
            t1 = wrk.tile([1, K], F32, tag="t1")
            nc.vector.tensor_scalar(
                out=t1[:],
                in0=u_ps[:],
                scalar1=2.0,
                scalar2=sh1[:, d : d + 1],
                op0=mybir.AluOpType.mult,
                op1=mybir.AluOpType.subtract,
            )
            t2 = wrk.tile([1, K], F32, tag="t2")
            nc.vector.tensor_scalar_mul(t2[:], c2_d, -ntf)
            nc.vector.tensor_tensor(
                out=t1[:], in0=t1[:], in1=t2[:], op=mybir.AluOpType.add
            )
            nc.vector.tensor_tensor(
                out=sumr, in0=sumr, in1=t1[:], op=mybir.AluOpType.add
            )
            # f = (u*(-4) + 2sh_d - t2) * c2 ; e = 4*(Pq - w) ; ssum += e + f
            f = wrk.tile([1, K], F32, tag="f")
            nc.vector.tensor_scalar(
                out=f[:],
                in0=u_ps[:],
                scalar1=-4.0,
                scalar2=sh2[:, d : d + 1],
                op0=mybir.AluOpType.mult,
                op1=mybir.AluOpType.add,
            )
            nc.vector.tensor_tensor(
                out=f[:], in0=f[:], in1=t2[:], op=mybir.AluOpType.subtract
            )
            nc.vector.tensor_tensor(
                out=f[:], in0=f[:], in1=c2_d, op=mybir.AluOpType.mult
            )
            e = wrk.tile([1, K], F32, tag="e")
            nc.vector.tensor_scalar_mul(e[:], w_ps[:], -4.0)
            nc.vector.tensor_tensor(
                out=e[:], in0=e[:], in1=f[:], op=mybir.AluOpType.add
            )
            t3 = wrk.tile([1, K], F32, tag="t3")
            nc.vector.tensor_scalar_mul(t3[:], pq_ps[:], 4.0)
            nc.vector.tensor_tensor(
                out=e[:], in0=e[:], in1=t3[:], op=mybir.AluOpType.add
            )
            nc.vector.tensor_tensor(
                out=ssum, in0=ssum, in1=e[:], op=mybir.AluOpType.add
            )
        nc.vector.tensor_scalar(
            out=ssum,
            in0=ssum,
            scalar1=shhtot[:, 0:1],
            scalar2=None,
            op0=mybir.AluOpType.add,
        )
        nc.sync.dma_start(red_loc[0:1, :], sumr)
        nc.sync.dma_start(red_loc[1:2, :], ssum)
        nc.gpsimd.collective_compute(
            "AllReduce",
            mybir.AluOpType.add,
            replica_groups=[list(range(ncores))],
            ins=[red_loc[:]],
            outs=[red_sum[:]],
        )
        sumg = con.tile([1, K], F32)
        nc.sync.dma_start(sumg[:], red_sum[0:1, :])
        ssumg = con.tile([1, K], F32)
        nc.sync.dma_start(ssumg[:], red_sum[1:2, :])
        if debug:
            nc.sync.dma_start(dbg["dbg_g"][:], g_sb[:])
            nc.sync.dma_start(dbg["dbg_red"][0:1, :], sumg[:])
            nc.sync.dma_start(dbg["dbg_red"][1:2, :], ssumg[:])

        # mean, var, s
        inv_nd = 1.0 / float(nd_tot)
        mean = con.tile([1, K], F32)
        nc.vector.tensor_scalar_mul(mean[:], sumg[:], inv_nd)
        var = con.tile([1, K], F32)
        nc.vector.tensor_scalar_mul(var[:], ssumg[:], inv_nd)
        m2 = con.tile([1, K], F32)
        nc.vector.tensor_tensor(
            out=m2[:], in0=mean[:], in1=mean[:], op=mybir.AluOpType.mult
        )
        nc.vector.tensor_tensor(
            out=var[:], in0=var[:], in1=m2[:], op=mybir.AluOpType.subtract
        )
        if debug:
            nc.sync.dma_start(dbg["dbg_mean"][:], mean[:])
            nc.sync.dma_start(dbg["dbg_var"][:], var[:])
        nc.vector.tensor_scalar_add(var[:], var[:], BN_EPS)
        rec = con.tile([1, K], F32)
        nc.vector.reciprocal(rec[:], var[:])
        sca = con.tile([1, K], F32)
        nc.scalar.activation(sca[:], rec[:], mybir.ActivationFunctionType.Sqrt)
        nsca = con.tile([1, K], F32)
        nc.vector.tensor_scalar_mul(nsca[:], sca[:], -1.0)
        s2 = con.tile([1, K], F32)
        nc.vector.tensor_scalar_mul(s2[:], sca[:], 2.0)
        # materialized partition-broadcasts of the [1, K] rows
        # (outer product ones[SUB] x row[K] on the PE)
        ones_row = con.tile([1, SUB], F32)
        nc.gpsimd.memset(ones_row[:], 1.0)
        meanb = con.tile([SUB, K], F32)
        nscab = con.tile([SUB, K], F32)
        s2b = con.tile([SUB, K], F32)
        for src, dst in ((mean, meanb), (nsca, nscab), (s2, s2b)):
            bc_ps = pz.tile([SUB, K], F32, tag="za", name="bc_ps")
            nc.tensor.matmul(bc_ps[:], lhsT=ones_row[:], rhs=src[:])
            nc.scalar.activation(
                dst[:], bc_ps[:], mybir.ActivationFunctionType.Copy
            )
        # beta[d,k] = -(c2 + mean) * s
        beta = con.tile([D, K], F32)
        nc.vector.tensor_tensor(
            out=beta[:], in0=c2pd_sb[:], in1=meanb[0:D, :], op=mybir.AluOpType.add
        )
        nc.vector.tensor_tensor(
            out=beta[:], in0=beta[:], in1=nscab[0:D, :], op=mybir.AluOpType.mult
        )
        # caug[66, K] per d: rows 0:64 = 2*s*c^T, row 64 = beta, row 65 = -s
        caug = con.tile([AUG, D * K], F32)
        for d in range(D):
            nc.vector.tensor_tensor(
                out=caug[0:SUB, K * d : K * d + K],
                in0=ct_sb[:, K * d : K * d + K],
                in1=s2b[:],
                op=mybir.AluOpType.mult,
            )
            nc.sync.dma_start(
                caug[SUB : SUB + 1, K * d : K * d + K], beta[d : d + 1, :]
            )
            nc.sync.dma_start(
                caug[SUB + 1 : SUB + 2, K * d : K * d + K], nsca[0:1, :]
            )
        if debug:
            nc.sync.dma_start(dbg["dbg_caug"][:], caug[:])

        # ---- phase B: transpose, z, argmax ----
        for t in range(nt):
            xt = xa[t]
            xt_ps = [
                pxt.tile([AUG, 4 * P], F32, tag="xtp", name="xt_ps")
                for _ in range(2)
            ]
            for d in range(D):
                nc.tensor.transpose(
                    out=xt_ps[d // 4][:, P * (d % 4) : P * (d % 4) + P],
                    in_=xt[:, AUG * d : AUG * d + AUG],
                    identity=ident[:],
                )
            xt_sb = xts.tile([AUG, D * P], F32, tag="xtsb")
            nc.scalar.activation(
                xt_sb[:, 0 : 4 * P],
                xt_ps[0][:],
                mybir.ActivationFunctionType.Copy,
            )
            nc.vector.tensor_copy(xt_sb[:, 4 * P : 8 * P], xt_ps[1][:])
            zps = [
                pz.tile([P, 4 * K], F32, tag=tg, name="zps") for tg in ("za", "zb")
            ]
            for d in range(D):
                nc.tensor.matmul(
                    zps[d // 4][:, K * (d % 4) : K * (d % 4) + K],
                    lhsT=xt_sb[:, P * d : P * d + P],
                    rhs=caug[:, K * d : K * d + K],
                )
            # argmax via prefix-max scan: k* = sum_k 1[pscan_k < rowmax],
            # rowmax = pscan[:, K-1]. Scan on DVE, sign+accumulate on ACT.
            acc = msk.tile([P, D], F32, tag="acc")
            for d in range(D):
                pscan = msk.tile([P, K], F32, tag="pscan")
                nc.vector.tensor_tensor_scan(
                    out=pscan[:],
                    data0=zps[d // 4][:, K * (d % 4) : K * (d % 4) + K],
                    data1=zf_sb[:],
                    initial=-1e30,
                    op0=mybir.AluOpType.max,
                    op1=mybir.AluOpType.bypass,
                )
                dum = msk.tile([P, K], BF16, tag="dum")
                nc.scalar.activation(
                    dum[:],
                    pscan[:],
                    mybir.ActivationFunctionType.Sign,
                    bias=pscan[:, K - 1 : K],
                    scale=-1.0,
                    accum_out=acc[:, d : d + 1],
                )
                if debug and t == 0 and d == 0:
                    ztmp = msk.tile([P, K], F32, tag="ztmp", name="ztmp")
                    nc.vector.tensor_copy(ztmp[:], zps[0][:, 0:K])
                    nc.sync.dma_start(dbg["dbg_z00"][:], ztmp[:])
            nc.sync.dma_start(out[P * t : P * t + P, :], acc[:])

    return nc


def prep_host(centroids):
    """Host-side layout prep (pure functions of the weights)."""
    ct = np.ascontiguousarray(
        centroids.transpose(0, 2, 1)
        .reshape(D, SUB, K)
        .transpose(1, 0, 2)
        .reshape(SUB, D * K)
    )
    # ct[s, d*K + k] = centroids[d, k, s]
    c2pd = np.sum(centroids.astype(np.float64) ** 2, axis=-1).astype(np.float32)
    return dict(ct=ct, c2pd=c2pd)


def make_in_maps(inputs, query_wemb, centroids, ncores):
    common = prep_host(np.asarray(centroids, dtype=np.float32))
    ids_all = np.asarray(inputs, dtype=np.int32).reshape(-1)
    npc = ids_all.size // ncores
    wemb = np.asarray(query_wemb, dtype=np.float32)
    xg_all = wemb[ids_all]  # host-side gather: [N, 512]
    in_maps = []
    for c in range(ncores):
        in_maps.append({**common, "xg": xg_all[c * npc : (c + 1) * npc]})
    return in_maps, npc


_CACHE = {}


def kernel(inputs, query_wemb, centroids):
    from concourse.bass_utils import run_bass_kernel_spmd

    inputs = np.asarray(inputs)
    ncores = 8
    in_maps, npc = make_in_maps(inputs, query_wemb, centroids, ncores)
    key = (npc, ncores)
    if key not in _CACHE:
        _CACHE[key] = _hoist_excess_waits(build(npc, ncores))
    nc = _CACHE[key]
    res = run_bass_kernel_spmd(nc, in_maps, list(range(ncores)))
    codes = np.concatenate(
        [res.results[c]["out"] for c in range(ncores)], axis=0
    )  # [N, D] float32 exact integers
    codes = np.rint(codes).astype(np.int64)
    cent = np.asarray(centroids, dtype=np.float32)  # [D, K, SUB]
    full = cent[np.arange(D)[None, :], codes]  # [N, D, SUB]
    return full.reshape(inputs.shape + (EMB,)).astype(np.float32)


# revision 11
# speedup vs baseline: 29.8123x; 1.1551x over previous
"""DPQ embedding (vq_codebook) Trainium2 kernel, v3.

Computes, for inputs ids[32,2048], query_wemb[100000,512], centroids[8,256,64]:
  x = wemb[ids]  -> [N, 8, 64]
  response[n,d,k] = -||x_nd||^2 + 2 x_nd.c_dk - ||c_dk||^2
  BN over (n,d) per k (training stats), argmax_k, gather centroids -> [N, 512]

Sharding strategy (v3): data-parallel over tokens on 8 cores. The host
performs the embedding-row gather (it holds both ids and the table), so each
core uploads only the 8192x512 fp32 rows it needs (17 MB/core, 134 MB total)
instead of a replicated 211 MB table per core (1.7 GB) -- host->device I/O is
the wall-clock bottleneck under this axon runtime. On device, a strided DMA
widens each row tile into the augmented layout [x(64) | 1 | h] per subspace;
h is filled on device. BN statistics are computed exactly via per-subspace
Gram matrices G_d = Y_d^T Y_d accumulated on the PE across all tiles (8
accumulation groups packed into 2 PSUM banks, overlapping the input DMAs),
reduced to per-k sums locally, and AllReduced as a single [2, 256] tensor
(2 KB, vs 139 KB in v2). Normalized responses z = s_k*(r - m_k) come from one
fp32 matmul per (tile, d) with augmented 66-row centroid matrices
(scale/beta/h-coefficient folded in); argmax via a DVE prefix-max scan + ACT
Sign-with-accumulator counting strict prefixes below the row max
(first-occurrence argmax, exact in fp32); the tiny code tensor [N, 8] is
returned and the final centroid row lookup happens on host. The
straight-through estimator (out - x) + x is the identity in the forward pass
and is omitted.

A post-scheduling pass (_hoist_excess_waits) splits semaphore waits onto
standalone EventSemaphore instructions because this walrus build rejects >1
sync-wait command per compute instruction and any wait on a Drain.
"""

import os
import sys

for _p in ("/opt/trn_rl_repo", "/root/.axon_site/_ro/trn_rl_repo"):
    if os.path.isdir(_p) and _p not in sys.path:
        sys.path.insert(0, _p)
        break

from contextlib import ExitStack

import numpy as np

import concourse.bass as bass
import concourse.tile as tile
from concourse import mybir
from concourse.masks import make_identity

VOCAB = 100000
EMB = 512
D = 8
K = 256
SUB = 64
AUG = SUB + 2  # 66: [x(64) | ones | h]
WAUG = D * AUG  # 528
BN_EPS = 1e-3
P = 128

F32 = mybir.dt.float32
BF16 = mybir.dt.bfloat16
I32 = mybir.dt.int32


def _hoist_excess_waits(nc, cap=1):
    """This walrus build rejects instructions carrying too many sync-wait
    commands (and any wait on a Drain). Hoist excess waits into standalone
    InstEventSemaphore instructions right before the offender, same engine."""
    uid = 0
    for f in nc.m.functions:
        for b in f.blocks:
            insts = b.instructions
            i = 0
            while i < len(insts):
                inst = insts[i]
                si = inst.sync_info
                if si is not None and si.on_wait:
                    c = 0 if type(inst).__name__ == "InstDrain" else cap
                    waits = list(si.on_wait)
                    if len(waits) > c:
                        nh = len(waits) - c
                        for w in waits[:nh]:
                            uid += 1
                            ev = mybir.InstEventSemaphore(
                                name=f"EVW-{uid}",
                                engine=inst.engine,
                                ins=[],
                                outs=[],
                                sync_info=mybir.SyncInfo(on_wait=[w], on_update=[]),
                            )
                            insts.insert(i, ev)
                            i += 1
                        inst.sync_info = mybir.SyncInfo(
                            on_wait=waits[nh:], on_update=list(si.on_update)
                        )
                i += 1
    return nc


def build(npc, ncores, debug=False):
    """Build the SPMD Bass program for `npc` tokens per core."""
    nt = npc // P  # token tiles per core
    nd_tot = npc * ncores * D  # BN sample count

    nc = bass.Bass()
    dbg = {}
    if debug:
        for nm, shp in [
            ("dbg_g", [AUG, WAUG]),
            ("dbg_red", [2, K]),
            ("dbg_mean", [1, K]),
            ("dbg_var", [1, K]),
            ("dbg_caug", [AUG, D * K]),
            ("dbg_z00", [P, K]),
        ]:
            dbg[nm] = nc.dram_tensor(nm, shp, F32, kind="ExternalOutput")

    xg = nc.dram_tensor("xg", [npc, EMB], F32, kind="ExternalInput")
    ct = nc.dram_tensor("ct", [SUB, D * K], F32, kind="ExternalInput")
    c2pd = nc.dram_tensor("c2pd", [D, K], F32, kind="ExternalInput")
    out = nc.dram_tensor("out", [npc, D], F32, kind="ExternalOutput")

    red_loc = nc.dram_tensor("red_loc", [2, K], F32)
    red_sum = nc.dram_tensor(
        "red_sum", [2, K], F32, addr_space="Shared" if ncores > 4 else "Local"
    )

    with ExitStack() as ctx:
        tc = ctx.enter_context(tile.TileContext(nc))
        con = ctx.enter_context(tc.tile_pool(name="con", bufs=1))
        xap = ctx.enter_context(tc.tile_pool(name="xap", bufs=1))
        wrk = ctx.enter_context(tc.tile_pool(name="wrk", bufs=2))
        xts = ctx.enter_context(tc.tile_pool(name="xts", bufs=3))
        msk = ctx.enter_context(tc.tile_pool(name="msk", bufs=3))
        pg = ctx.enter_context(tc.tile_pool(name="pg", bufs=1, space="PSUM"))
        pxt = ctx.enter_context(tc.tile_pool(name="pxt", bufs=2, space="PSUM"))
        pz = ctx.enter_context(tc.tile_pool(name="pz", bufs=1, space="PSUM"))

        # ---- constants / small inputs ----
        ident = con.tile([P, P], F32)
        make_identity(nc, ident[:])
        ones64 = con.tile([SUB, 1], F32)
        nc.gpsimd.memset(ones64[:], 1.0)
        c2pd_sb = con.tile([D, K], F32)
        nc.sync.dma_start(c2pd_sb[:], c2pd[:])
        ct_sb = con.tile([SUB, D * K], F32)
        nc.sync.dma_start(ct_sb[:], ct[:])
        zf_sb = con.tile([P, K], F32)
        nc.gpsimd.memset(zf_sb[:], 0.0)

        # ---- phase A: widen-load + h, then d-major Gram accumulation ----
        # (PSUM accumulation groups own a full 2KB zero region, so the 8
        # Gram groups must run sequentially; the d=0 sweep visits tiles in
        # DMA arrival order, so it still overlaps the input loads.)
        xa = []
        for t in range(nt):
            xt = xap.tile([P, WAUG], F32, tag=f"xa{t}")
            xa.append(xt)
            xv3 = xt[:].rearrange("p (d c) -> p d c", c=AUG)
            nc.sync.dma_start(
                xv3[:, :, 0:SUB],
                xg[P * t : P * t + P, :].rearrange("p (d c) -> p d c", c=SUB),
            )
            nc.gpsimd.memset(xv3[:, :, SUB : SUB + 1], 1.0)
            xv = xv3[:, :, 0:SUB]
            x2 = wrk.tile([P, D * SUB], F32, tag="x2")
            x2v = x2[:].rearrange("p (d c) -> p d c", c=SUB)
            nc.gpsimd.tensor_tensor(out=x2v, in0=xv, in1=xv, op=mybir.AluOpType.mult)
            htmp = wrk.tile([P, D], F32, tag="htmp")
            nc.vector.tensor_reduce(
                out=htmp[:], in_=x2v, axis=mybir.AxisListType.X, op=mybir.AluOpType.add
            )
            nc.vector.tensor_copy(xv3[:, :, SUB + 1 : SUB + 2], htmp[:])

        g_sb = con.tile([AUG, WAUG], F32)
        for d in range(D):
            gp = pg.tile([AUG, AUG], F32, tag=f"gb{d % 2}", name="gp")
            for t in range(nt):
                nc.tensor.matmul(
                    gp[:],
                    lhsT=xa[t][:, AUG * d : AUG * d + AUG],
                    rhs=xa[t][:, AUG * d : AUG * d + AUG],
                    start=(t == 0),
                    stop=(t == nt - 1),
                )
            nc.scalar.activation(
                g_sb[:, AUG * d : AUG * d + AUG],
                gp[:],
                mybir.ActivationFunctionType.Copy,
            )

        # ---- local BN sums, then a tiny AllReduce ----
        # per-d scalars: sh_d = sum h, shh_d = sum h^2 (k-free)
        gv = g_sb[:].rearrange("p (d c) -> p d c", c=AUG)
        sh1 = con.tile([1, D], F32)
        nc.sync.dma_start(sh1[:], gv[SUB : SUB + 1, :, SUB + 1 : SUB + 2])
        shh1 = con.tile([1, D], F32)
        nc.sync.dma_start(shh1[:], gv[SUB + 1 : SUB + 2, :, SUB + 1 : SUB + 2])
        shhtot = con.tile([1, 1], F32)
        nc.vector.reduce_sum(shhtot[:], shh1[:], axis=mybir.AxisListType.X)

        sh2 = con.tile([1, D], F32)
        nc.vector.tensor_scalar_mul(sh2[:], sh1[:], 2.0)

        ntf = float(npc)  # LOCAL token count (sums are AllReduced below)
        # accumulate over d into red2 rows:
        #   row0 sumr[k] += 2*u_dk - NT*c2_dk - sh_d
        #   row1 ssum[k] += 4*(Pq_dk - w_dk) + c2_dk*(NT*c2_dk - 4*u_dk + 2*sh_d)
        sumr_t = con.tile([1, K], F32)
        ssum_t = con.tile([1, K], F32)
        sumr = sumr_t[:]
        ssum = ssum_t[:]
        nc.gpsimd.memset(sumr, 0.0)
        nc.gpsimd.memset(ssum, 0.0)
        for d in range(D):
            ct_d = ct_sb[:, K * d : K * d + K]
            c2_t = wrk.tile([1, K], F32, tag="c2d")
            nc.sync.dma_start(c2_t[:], c2pd[d : d + 1, :])
            c2_d = c2_t[:]
            t_ps = pz.tile([SUB, K], F32, tag="za", name="t_ps")
            nc.tensor.matmul(
                t_ps[:], lhsT=g_sb[0:SUB, AUG * d : AUG * d + SUB], rhs=ct_d
            )
            m_sb = wrk.tile([SUB, K], F32, tag="msb")
            nc.vector.tensor_tensor(
                out=m_sb[:], in0=ct_d, in1=t_ps[:], op=mybir.AluOpType.mult
            )
            pq_ps = pz.tile([1, K], F32, tag="zb", name="pq_ps")
            nc.tensor.matmul(pq_ps[:], lhsT=ones64[:], rhs=m_sb[:])
            u_ps = pz.tile([1, K], F32, tag="za", name="u_ps")
            nc.tensor.matmul(
                u_ps[:],
                lhsT=g_sb[0:SUB, AUG * d + SUB : AUG * d + SUB + 1],
                rhs=ct_d,
            )
            w_ps = pz.tile([1, K], F32, tag="zb", name="w_ps")
            nc.tensor.matmul(
                w_ps[:],
                lhsT=g_sb[0:SUB, AUG * d + SUB + 1 : AUG * d + AUG],
                rhs=ct_d,
            )
            # sumr += (u*2 - sh_d) + c2*(-NT)
            t1 = wrk.tile([1, K], F32, tag="t1")
            nc.vector.tensor_scalar(
                out=t1[:],
                in0=u_ps[:],
                scalar1=2.0,
                scalar2=sh1[:, d : d + 1],
                op0=mybir.AluOpType.mult,
                op1=mybir.AluOpType.subtract,
            )
            t2 = wrk.tile([1, K], F32, tag="t2")
            nc.vector.tensor_scalar_mul(t2[:], c2_d, -ntf)
            nc.vector.tensor_tensor(
                out=t1[:], in0=t1[:], in1=t2[:], op=mybir.AluOpType.add
            )
            nc.vector.tensor_tensor(
                out=sumr, in0=sumr, in1=t1[:], op=mybir.AluOpType.add
            )
            # f = (u*(-4) + 2sh_d - t2) * c2 ; e = 4*(Pq - w) ; ssum += e + f
            f = wrk.tile([1, K], F32, tag="f")
            nc.vector.tensor_scalar(
                out=f[:],
                in0=u_ps[:],
                scalar1=-4.0,
                scalar2=sh2[:, d : d + 1],
                op0=mybir.AluOpType.mult,
                op1=mybir.AluOpType.add,
            )
            nc.vector.tensor_tensor(
                out=f[:], in0=f[:], in1=t2[:], op=mybir.AluOpType.subtract
            )
            nc.vector.tensor_tensor(
                out=f[:], in0=f[:], in1=c2_d, op=mybir.AluOpType.mult
            )
            e = wrk.tile([1, K], F32, tag="e")
            nc.vector.tensor_scalar_mul(e[:], w_ps[:], -4.0)
            nc.vector.tensor_tensor(
                out=e[:], in0=e[:], in1=f[:], op=mybir.AluOpType.add
            )
            t3 = wrk.tile([1, K], F32, tag="t3")
            nc.vector.tensor_scalar_mul(t3[:], pq_ps[:], 4.0)
            nc.vector.tensor_tensor(
                out=e[:], in0=e[:], in1=t3[:], op=mybir.AluOpType.add
            )
            nc.vector.tensor_tensor(
                out=ssum, in0=ssum, in1=e[:], op=mybir.AluOpType.add
            )
        nc.vector.tensor_scalar(
            out=ssum,
            in0=ssum,
            scalar1=shhtot[:, 0:1],
            scalar2=None,
            op0=mybir.AluOpType.add,
        )
        nc.sync.dma_start(red_loc[0:1, :], sumr)
        nc.sync.dma_start(red_loc[1:2, :], ssum)
        nc.gpsimd.collective_compute(
            "AllReduce",
            mybir.AluOpType.add,
            replica_groups=[list(range(ncores))],
            ins=[red_loc[:]],
            outs=[red_sum[:]],
        )
        sumg = con.tile([1, K], F32)
        nc.sync.dma_start(sumg[:], red_sum[0:1, :])
        ssumg = con.tile([1, K], F32)
        nc.sync.dma_start(ssumg[:], red_sum[1:2, :])
        if debug:
            nc.sync.dma_start(dbg["dbg_g"][:], g_sb[:])
            nc.sync.dma_start(dbg["dbg_red"][0:1, :], sumg[:])
            nc.sync.dma_start(dbg["dbg_red"][1:2, :], ssumg[:])

        # mean, var, s
        inv_nd = 1.0 / float(nd_tot)
        mean = con.tile([1, K], F32)
        nc.vector.tensor_scalar_mul(mean[:], sumg[:], inv_nd)
        var = con.tile([1, K], F32)
        nc.vector.tensor_scalar_mul(var[:], ssumg[:], inv_nd)
        m2 = con.tile([1, K], F32)
        nc.vector.tensor_tensor(
            out=m2[:], in0=mean[:], in1=mean[:], op=mybir.AluOpType.mult
        )
        nc.vector.tensor_tensor(
            out=var[:], in0=var[:], in1=m2[:], op=mybir.AluOpType.subtract
        )
        if debug:
            nc.sync.dma_start(dbg["dbg_mean"][:], mean[:])
            nc.sync.dma_start(dbg["dbg_var"][:], var[:])
        nc.vector.tensor_scalar_add(var[:], var[:], BN_EPS)
        rec = con.tile([1, K], F32)
        nc.vector.reciprocal(rec[:], var[:])
        sca = con.tile([1, K], F32)
        nc.scalar.activation(sca[:], rec[:], mybir.ActivationFunctionType.Sqrt)
        nsca = con.tile([1, K], F32)
        nc.vector.tensor_scalar_mul(nsca[:], sca[:], -1.0)
        s2 = con.tile([1, K], F32)
        nc.vector.tensor_scalar_mul(s2[:], sca[:], 2.0)
        # materialized partition-broadcasts of the [1, K] rows
        # (outer product ones[SUB] x row[K] on the PE)
        ones_row = con.tile([1, SUB], F32)
        nc.gpsimd.memset(ones_row[:], 1.0)
        meanb = con.tile([SUB, K], F32)
        nscab = con.tile([SUB, K], F32)
        s2b = con.tile([SUB, K], F32)
        for src, dst in ((mean, meanb), (nsca, nscab), (s2, s2b)):
            bc_ps = pz.tile([SUB, K], F32, tag="za", name="bc_ps")
            nc.tensor.matmul(bc_ps[:], lhsT=ones_row[:], rhs=src[:])
            nc.scalar.activation(
                dst[:], bc_ps[:], mybir.ActivationFunctionType.Copy
            )
        # beta[d,k] = -(c2 + mean) * s
        beta = con.tile([D, K], F32)
        nc.vector.tensor_tensor(
            out=beta[:], in0=c2pd_sb[:], in1=meanb[0:D, :], op=mybir.AluOpType.add
        )
        nc.vector.tensor_tensor(
            out=beta[:], in0=beta[:], in1=nscab[0:D, :], op=mybir.AluOpType.mult
        )
        # caug[66, K] per d: rows 0:64 = 2*s*c^T, row 64 = beta, row 65 = -s
        caug = con.tile([AUG, D * K], F32)
        for d in range(D):
            nc.vector.tensor_tensor(
                out=caug[0:SUB, K * d : K * d + K],
                in0=ct_sb[:, K * d : K * d + K],
                in1=s2b[:],
                op=mybir.AluOpType.mult,
            )
            nc.sync.dma_start(
                caug[SUB : SUB + 1, K * d : K * d + K], beta[d : d + 1, :]
            )
            nc.sync.dma_start(
                caug[SUB + 1 : SUB + 2, K * d : K * d + K], nsca[0:1, :]
            )
        if debug:
            nc.sync.dma_start(dbg["dbg_caug"][:], caug[:])

        # ---- phase B: transpose, z, argmax ----
        for t in range(nt):
            xt = xa[t]
            xt_ps = [
                pxt.tile([AUG, 4 * P], F32, tag="xtp", name="xt_ps")
                for _ in range(2)
            ]
            for d in range(D):
                nc.tensor.transpose(
                    out=xt_ps[d // 4][:, P * (d % 4) : P * (d % 4) + P],
                    in_=xt[:, AUG * d : AUG * d + AUG],
                    identity=ident[:],
                )
            xt_sb = xts.tile([AUG, D * P], F32, tag="xtsb")
            nc.scalar.activation(
                xt_sb[:, 0 : 4 * P],
                xt_ps[0][:],
                mybir.ActivationFunctionType.Copy,
            )
            nc.vector.tensor_copy(xt_sb[:, 4 * P : 8 * P], xt_ps[1][:])
            zps = [
                pz.tile([P, 4 * K], F32, tag=tg, name="zps") for tg in ("za", "zb")
            ]
            for d in range(D):
                nc.tensor.matmul(
                    zps[d // 4][:, K * (d % 4) : K * (d % 4) + K],
                    lhsT=xt_sb[:, P * d : P * d + P],
                    rhs=caug[:, K * d : K * d + K],
                )
            # argmax via prefix-max scan: k* = sum_k 1[pscan_k < rowmax],
            # rowmax = pscan[:, K-1]. Scan on DVE, sign+accumulate on ACT.
            acc = msk.tile([P, D], F32, tag="acc")
            for d in range(D):
                pscan = msk.tile([P, K], F32, tag="pscan")
                nc.vector.tensor_tensor_scan(
                    out=pscan[:],
                    data0=zps[d // 4][:, K * (d % 4) : K * (d % 4) + K],
                    data1=zf_sb[:],
                    initial=-1e30,
                    op0=mybir.AluOpType.max,
                    op1=mybir.AluOpType.bypass,
                )
                dum = msk.tile([P, K], BF16, tag="dum")
                nc.scalar.activation(
                    dum[:],
                    pscan[:],
                    mybir.ActivationFunctionType.Sign,
                    bias=pscan[:, K - 1 : K],
                    scale=-1.0,
                    accum_out=acc[:, d : d + 1],
                )
                if debug and t == 0 and d == 0:
                    ztmp = msk.tile([P, K], F32, tag="ztmp", name="ztmp")
                    nc.vector.tensor_copy(ztmp[:], zps[0][:, 0:K])
                    nc.sync.dma_start(dbg["dbg_z00"][:], ztmp[:])
            nc.sync.dma_start(out[P * t : P * t + P, :], acc[:])

    return nc


def prep_host(centroids):
    """Host-side layout prep (pure functions of the weights)."""
    ct = np.ascontiguousarray(
        centroids.transpose(0, 2, 1)
        .reshape(D, SUB, K)
        .transpose(1, 0, 2)
        .reshape(SUB, D * K)
    )
    # ct[s, d*K + k] = centroids[d, k, s]
    c2pd = np.sum(centroids.astype(np.float64) ** 2, axis=-1).astype(np.float32)
    return dict(ct=ct, c2pd=c2pd)


def make_in_maps(inputs, query_wemb, centroids, ncores):
    common = prep_host(np.asarray(centroids, dtype=np.float32))
    ids_all = np.asarray(inputs, dtype=np.int32).reshape(-1)
    npc = ids_all.size // ncores
    wemb = np.asarray(query_wemb, dtype=np.float32)
    xg_all = wemb[ids_all]  # host-side gather: [N, 512]
    in_maps = []
    for c in range(ncores):
        in_maps.append({**common, "xg": xg_all[c * npc : (c + 1) * npc]})
    return in_maps, npc


_CACHE = {}


def kernel(inputs, query_wemb, centroids):
    from concourse.bass_utils import run_bass_kernel_spmd

    inputs = np.asarray(inputs)
    ncores = 8
    in_maps, npc = make_in_maps(inputs, query_wemb, centroids, ncores)
    key = (npc, ncores)
    if key not in _CACHE:
        _CACHE[key] = _hoist_excess_waits(build(npc, ncores))
    nc = _CACHE[key]
    res = run_bass_kernel_spmd(nc, in_maps, list(range(ncores)))
    codes = np.concatenate(
        [res.results[c]["out"] for c in range(ncores)], axis=0
    )  # [N, D] float32 exact integers
    codes = np.rint(codes).astype(np.int64)
    codes += np.arange(D, dtype=np.int64)[None, :] * K
    cent2 = np.ascontiguousarray(centroids, dtype=np.float32).reshape(D * K, SUB)
    full = cent2.take(codes.ravel(), axis=0)  # [N*D, SUB] fp32
    return full.reshape(inputs.shape + (EMB,))
